# revision 14
# baseline (speedup 1.0000x reference)
"""Dense-MoE (top-2 of 8 experts) TRN2 kernel: expert-parallel over 8 NeuronCores.

Host side: softmax + top-2 routing, per-expert token gather (padded to the max
expert load), weight re-layout, fp32->bf16 cast. Device side (per core = one
expert), all matmul operands bf16 (PSUM fp32):
    h = silu(x_e @ gw.T) * (x_e @ uw.T)        [F-major in SBUF, bf16]
    out_e[d, t] = sum_f dwT[f, d] * h[f, t]     [dw stationary, tokens moving]
Host applies the routing weights and scatter-adds the 8 per-expert outputs
into the [T, D] result.

bf16 keeps absmax rel err ~4e-3 (vs the 2e-2 gate) while halving DMA traffic;
tokens-moving phase B avoids paying a full 512-col PE pass for the partial
last token tile.
"""
import sys

sys.path.insert(0, "/opt/trn_rl_repo")

import numpy as np
import ml_dtypes

import concourse.bass as bass
from concourse import bacc
import concourse.mybir as mybir
import concourse.tile as tile
from concourse.bass_utils import run_bass_kernel_spmd
from concourse.bass import ds

T, D, F, E, TOPK = 4096, 1024, 2048, 8, 2
P = 128
N_CORES = 8

F32 = mybir.dt.float32
BF16 = mybir.dt.bfloat16
NP_BF16 = ml_dtypes.bfloat16


def _chunks(cap):
    """Balanced even token chunks <=512."""
    n = -(-cap // 512)
    base = (cap // n) & ~1
    sizes = [base] * n
    rem = cap - base * n
    i = 0
    while rem > 0:
        sizes[i % n] += 2
        rem -= 2
        i += 1
    out = []
    c0 = 0
    for cs in sizes:
        out.append((c0, cs))
        c0 += cs
    return out


def _build(cap):
    """Per-core Bass program for capacity `cap` (even) tokens."""
    assert cap % 2 == 0
    achunks = _chunks(cap)            # phase A token chunks
    # Phase B: big chunks first, smaller final chunk so the end-of-kernel
    # copy+DMA drain is short. Keep >=256 so the per-group PE time stays
    # ahead of the copy + DMA-enqueue consumer chain (~1us per group).
    if cap > 1024:
        last = 256
        mid = cap - 512 - last
        bchunks = [(0, 512), (512, mid), (512 + mid, last)]
    else:
        bchunks = _chunks(cap)
    n_dt = D // P                     # 8 contraction slices
    n_ft = F // P                     # 16 f tiles
    n_do = D // P                     # 8 output d tiles

    nc = bacc.Bacc(None, target_bir_lowering=False)
    x_d = nc.declare_dram_parameter("x", [P, n_dt, cap], BF16, isOutput=False)
    gw_d = nc.declare_dram_parameter("gw", [P, n_ft, n_dt, P], BF16, isOutput=False)
    uw_d = nc.declare_dram_parameter("uw", [P, n_ft, n_dt, P], BF16, isOutput=False)
    # First f-tile of gate/up duplicated in tile layout (2KB/partition
    # contiguous) so the opening loads are single efficient DMAs.
    gw0_d = nc.declare_dram_parameter("gw0", [P, n_dt, P], BF16, isOutput=False)
    uw0_d = nc.declare_dram_parameter("uw0", [P, n_dt, P], BF16, isOutput=False)
    dw_d = nc.declare_dram_parameter("dw", [P, n_ft, D], BF16, isOutput=False)
    out_d = nc.declare_dram_parameter("out", [n_do, P, cap], BF16, isOutput=True)

    with tile.TileContext(nc) as tc:
        with (
            tc.tile_pool(name="deep", bufs=1) as deep,
            tc.tile_pool(name="wts", bufs=3) as wts,
            tc.tile_pool(name="stage", bufs=2) as stage,
            tc.tile_pool(name="ps", bufs=2, space="PSUM") as ps,
        ):
            wt_tiles = {}

            # PE warmup: dummy matmuls on a memset scratch tile. They run
            # during the opening DMA wait (no data deps) and pre-ramp the
            # HAM clock gate, so the first real matmuls start near 2.4GHz
            # instead of 1.2GHz.
            scr = stage.tile([P, 512], BF16, tag="scr")
            nc.vector.memset(scr[:], 0.0)
            for _ in range(2):
                pw = ps.tile([P, 512], F32, tag="pg")
                nc.tensor.matmul(pw[:], scr[:, :P], scr[:], start=True,
                                 stop=True, skip_group_check=True)

            # Prologue. Each dma_start costs ~600ns of serial sequencer
            # enqueue time (per call, not per run), so x goes on the
            # otherwise-idle gpsimd lane as 3 fused multi-run DMAs while the
            # sync lane enqueues weights in parallel.
            c0s = achunks[0][1]
            gw0 = wts.tile([P, n_dt, P], BF16, tag="gw")
            nc.sync.dma_start(gw0[:], gw0_d[:])
            x_t = deep.tile([P, n_dt, cap], BF16, tag="x")
            nc.gpsimd.dma_start(x_t[:, ds(0, 4), ds(0, c0s)],
                                x_d[:, ds(0, 4), ds(0, c0s)])
            nc.gpsimd.dma_start(x_t[:, ds(4, 4), ds(0, c0s)],
                                x_d[:, ds(4, 4), ds(0, c0s)])
            nc.gpsimd.dma_start(x_t[:, :, ds(c0s, cap - c0s)],
                                x_d[:, :, ds(c0s, cap - c0s)])
            uw0 = wts.tile([P, n_dt, P], BF16, tag="uw")
            nc.sync.dma_start(uw0[:], uw0_d[:])
            wt_tiles[0] = (gw0, uw0)

            h_t = deep.tile([P, n_ft, cap], BF16, tag="h")
            dw_t = deep.tile([P, n_ft, D], BF16, tag="dw")

            def load_ft(ft):
                gw_t = wts.tile([P, n_dt, P], BF16, tag="gw")
                nc.sync.dma_start(gw_t[:], gw_d[:, ft])
                uw_t = wts.tile([P, n_dt, P], BF16, tag="uw")
                nc.sync.dma_start(uw_t[:], uw_d[:, ft])
                wt_tiles[ft] = (gw_t, uw_t)

            # Phase A: h[fp, ft, c] = silu(g) * u, F-major, bf16
            for ft in range(n_ft):
                if ft == 8:
                    # Down weights enqueued once phase A's own traffic has
                    # drained; ready well before phase B needs them.
                    nc.gpsimd.dma_start(dw_t[:], dw_d[:])
                if ft not in wt_tiles:
                    load_ft(ft)
                gw_t, uw_t = wt_tiles.pop(ft)
                for (c0, cs) in achunks:
                    pg = ps.tile([P, 512], F32, tag="pg")
                    for dt_ in range(n_dt):
                        nc.tensor.matmul(
                            pg[:, :cs], gw_t[:, dt_], x_t[:, dt_, ds(c0, cs)],
                            start=(dt_ == 0), stop=(dt_ == n_dt - 1),
                        )
                    pu = ps.tile([P, 512], F32, tag="pu")
                    for dt_ in range(n_dt):
                        nc.tensor.matmul(
                            pu[:, :cs], uw_t[:, dt_], x_t[:, dt_, ds(c0, cs)],
                            start=(dt_ == 0), stop=(dt_ == n_dt - 1),
                        )
                    sg = stage.tile([P, 512], F32, tag="sg")
                    nc.scalar.activation(sg[:, :cs], pg[:, :cs],
                                         mybir.ActivationFunctionType.Silu)
                    nc.vector.tensor_tensor(
                        h_t[:, ft, ds(c0, cs)], sg[:, :cs], pu[:, :cs],
                        mybir.AluOpType.mult,
                    )

            # Phase B: out[d, c] = sum_f dwT[f, d] * h[f, c]
            # dw stationary (always 128 cols -> FWL), tokens moving, so the
            # trailing partial token tile costs only its actual width.
            for (c0, cs) in bchunks:
                for do in range(n_do):
                    po = ps.tile([P, 512], F32, tag="po")
                    for fo in range(n_ft):
                        nc.tensor.matmul(
                            po[:, :cs], dw_t[:, fo, ds(do * P, P)],
                            h_t[:, fo, ds(c0, cs)],
                            start=(fo == 0), stop=(fo == n_ft - 1),
                        )
                    osb = stage.tile([P, 512], BF16, tag="osb")
                    nc.scalar.activation(osb[:, :cs], po[:, :cs],
                                         mybir.ActivationFunctionType.Copy)
                    nc.sync.dma_start(out_d[do, :, ds(c0, cs)], osb[:, :cs])
    nc.finalize()
    return nc


def _route(gating_output):
    """Numpy softmax + top-2 + renormalize; returns (ids [T,K], w [T,K])."""
    g = gating_output.astype(np.float32)
    m = g.max(axis=-1, keepdims=True)
    e = np.exp(g - m)
    probs = e / e.sum(axis=-1, keepdims=True)
    ids = np.argsort(-probs, axis=-1, kind="stable")[:, :TOPK]
    w = np.take_along_axis(probs, ids, axis=-1)
    w = w / w.sum(axis=-1, keepdims=True)
    return ids, w


def kernel(x, gating_output, gate_w, up_w, down_w):
    x = np.asarray(x, dtype=np.float32)
    gating_output = np.asarray(gating_output, dtype=np.float32)
    gate_w = np.asarray(gate_w, dtype=np.float32)
    up_w = np.asarray(up_w, dtype=np.float32)
    down_w = np.asarray(down_w, dtype=np.float32)

    ids, w = _route(gating_output)

    idx_e = []
    w_e = []
    for e in range(E):
        sel = np.nonzero((ids == e).any(axis=-1))[0]
        kpos = (ids[sel] == e).argmax(axis=-1)
        idx_e.append(sel)
        w_e.append(w[sel, kpos])

    cap = max(len(i) for i in idx_e)
    cap += cap & 1

    nc = _build(cap)

    in_maps = []
    for e in range(E):
        idx = idx_e[e]
        cnt = len(idx)
        x_pad = np.zeros((cap, D), dtype=np.float32)
        x_pad[:cnt] = x[idx]

        # x: [cap, D] -> [128(dp), D/128(dt), cap]
        x_dev = np.ascontiguousarray(
            x_pad.T.reshape(D // P, P, cap).transpose(1, 0, 2).astype(NP_BF16))
        # gate/up: [F, D] -> T -> [D, F] -> [128(dp), 16(ft), 8(dt), 128(fi)]
        gwT = gate_w[e].T  # [D, F]
        gw_dev = np.ascontiguousarray(
            gwT.reshape(D // P, P, F // P, P).transpose(1, 2, 0, 3).astype(NP_BF16))
        uwT = up_w[e].T
        uw_dev = np.ascontiguousarray(
            uwT.reshape(D // P, P, F // P, P).transpose(1, 2, 0, 3).astype(NP_BF16))
        # down: [D, F] -> T -> [F, D] -> [128(fp), 16(fo), D]
        dwT = down_w[e].T  # [F, D]
        dw_dev = np.ascontiguousarray(
            dwT.reshape(F // P, P, D).transpose(1, 0, 2).astype(NP_BF16))
        gw0_dev = np.ascontiguousarray(gw_dev[:, 0])
        uw0_dev = np.ascontiguousarray(uw_dev[:, 0])

        in_maps.append({"x": x_dev, "gw": gw_dev, "uw": uw_dev, "dw": dw_dev,
                        "gw0": gw0_dev, "uw0": uw0_dev})

    try:
        res = run_bass_kernel_spmd(nc, in_maps, core_ids=list(range(N_CORES)))
    except Exception:
        # First execution of a fresh NEFF occasionally dies with
        # NRT_EXEC_UNIT_UNRECOVERABLE on this setup; the retry reuses the
        # cached executable and goes through.
        import time as _time

        _time.sleep(5)
        res = run_bass_kernel_spmd(nc, in_maps, core_ids=list(range(N_CORES)))

    out = np.zeros((T, D), dtype=np.float32)
    for e in range(E):
        cnt = len(idx_e[e])
        oe = res.results[e]["out"].reshape(D, cap).astype(np.float32)  # [d, token]
        out[idx_e[e]] += oe.T[:cnt] * w_e[e][:, None]
    return out


# revision 17
# speedup vs baseline: 1.0225x; 1.0225x over previous
"""Dense-MoE (top-2 of 8 experts) TRN2 kernel: expert-parallel over 8 NeuronCores.

Host side: softmax + top-2 routing, per-expert token gather (padded to the max
expert load), weight re-layout, fp32->bf16 cast. Device side (per core = one
expert), all matmul operands bf16 (PSUM fp32):
    h = silu(x_e @ gw.T) * (x_e @ uw.T)        [F-major in SBUF, bf16]
    out_e[d, t] = sum_f dwT[f, d] * h[f, t]     [dw stationary, tokens moving]
Host applies the routing weights and scatter-adds the 8 per-expert outputs
into the [T, D] result.

bf16 keeps absmax rel err ~4e-3 (vs the 2e-2 gate) while halving DMA traffic;
tokens-moving phase B avoids paying a full 512-col PE pass for the partial
last token tile.
"""
import sys

sys.path.insert(0, "/opt/trn_rl_repo")

import numpy as np
import ml_dtypes

import concourse.bass as bass
from concourse import bacc
import concourse.mybir as mybir
import concourse.tile as tile
from concourse.bass_utils import run_bass_kernel_spmd
from concourse.bass import ds

T, D, F, E, TOPK = 4096, 1024, 2048, 8, 2
P = 128
N_CORES = 8

F32 = mybir.dt.float32
BF16 = mybir.dt.bfloat16
NP_BF16 = ml_dtypes.bfloat16


def _chunks(cap):
    """Balanced even token chunks <=512."""
    n = -(-cap // 512)
    base = (cap // n) & ~1
    sizes = [base] * n
    rem = cap - base * n
    i = 0
    while rem > 0:
        sizes[i % n] += 2
        rem -= 2
        i += 1
    out = []
    c0 = 0
    for cs in sizes:
        out.append((c0, cs))
        c0 += cs
    return out


def _build(cap):
    """Per-core Bass program for capacity `cap` (even) tokens."""
    assert cap % 2 == 0
    achunks = _chunks(cap)            # phase A token chunks
    # Phase B: big chunks first, smaller final chunk so the end-of-kernel
    # copy+DMA drain is short. Keep >=256 so the per-group PE time stays
    # ahead of the copy + DMA-enqueue consumer chain (~1us per group).
    if 1024 < cap <= 1280:
        last = 256
        mid = cap - 512 - last
        bchunks = [(0, 512), (512, mid), (512 + mid, last)]
    else:
        bchunks = _chunks(cap)
    n_dt = D // P                     # 8 contraction slices
    n_ft = F // P                     # 16 f tiles
    n_do = D // P                     # 8 output d tiles

    nc = bacc.Bacc(None, target_bir_lowering=False)
    x_d = nc.declare_dram_parameter("x", [P, n_dt, cap], BF16, isOutput=False)
    gw_d = nc.declare_dram_parameter("gw", [P, n_ft, n_dt, P], BF16, isOutput=False)
    uw_d = nc.declare_dram_parameter("uw", [P, n_ft, n_dt, P], BF16, isOutput=False)
    # First f-tile of gate/up duplicated in tile layout (2KB/partition
    # contiguous) so the opening loads are single efficient DMAs.
    gw0_d = nc.declare_dram_parameter("gw0", [P, n_dt, P], BF16, isOutput=False)
    uw0_d = nc.declare_dram_parameter("uw0", [P, n_dt, P], BF16, isOutput=False)
    dw_d = nc.declare_dram_parameter("dw", [P, n_ft, D], BF16, isOutput=False)
    out_d = nc.declare_dram_parameter("out", [n_do, P, cap], BF16, isOutput=True)

    with tile.TileContext(nc) as tc:
        with (
            tc.tile_pool(name="deep", bufs=1) as deep,
            tc.tile_pool(name="wts", bufs=3) as wts,
            tc.tile_pool(name="stage", bufs=2) as stage,
            tc.tile_pool(name="ps", bufs=2, space="PSUM") as ps,
        ):
            wt_tiles = {}

            # PE warmup: dummy matmuls on a memset scratch tile. They run
            # during the opening DMA wait (no data deps) and pre-ramp the
            # HAM clock gate, so the first real matmuls start near 2.4GHz
            # instead of 1.2GHz.
            scr = stage.tile([P, 512], BF16, tag="scr")
            nc.vector.memset(scr[:], 0.0)
            for _ in range(2):
                pw = ps.tile([P, 512], F32, tag="pg")
                nc.tensor.matmul(pw[:], scr[:, :P], scr[:], start=True,
                                 stop=True, skip_group_check=True)

            # Prologue, all on the sync lane (gpsimd dma_start takes the slow
            # dynamic-DMA path). Each dma_start costs ~600ns of serial
            # sequencer enqueue time (per call, not per run), so x is loaded
            # as a few fused multi-run DMAs, split so each piece lands just
            # before its consumer needs it.
            c0s = achunks[0][1]
            gw0 = wts.tile([P, n_dt, P], BF16, tag="gw")
            nc.sync.dma_start(gw0[:], gw0_d[:])
            x_t = deep.tile([P, n_dt, cap], BF16, tag="x")
            nc.sync.dma_start(x_t[:, ds(0, 2), ds(0, c0s)],
                              x_d[:, ds(0, 2), ds(0, c0s)])
            nc.sync.dma_start(x_t[:, ds(2, 6), ds(0, c0s)],
                              x_d[:, ds(2, 6), ds(0, c0s)])
            uw0 = wts.tile([P, n_dt, P], BF16, tag="uw")
            if len(achunks) > 1:
                c1, c1s = achunks[1]
                nc.sync.dma_start(x_t[:, :, ds(c1, c1s)], x_d[:, :, ds(c1, c1s)])
                nc.sync.dma_start(uw0[:], uw0_d[:])
                for (cr, crs) in achunks[2:]:
                    nc.sync.dma_start(x_t[:, :, ds(cr, crs)], x_d[:, :, ds(cr, crs)])
            else:
                nc.sync.dma_start(uw0[:], uw0_d[:])
            wt_tiles[0] = (gw0, uw0)

            h_t = deep.tile([P, n_ft, cap], BF16, tag="h")
            dw_t = deep.tile([P, n_ft, D], BF16, tag="dw")

            def load_ft(ft):
                gw_t = wts.tile([P, n_dt, P], BF16, tag="gw")
                nc.sync.dma_start(gw_t[:], gw_d[:, ft])
                uw_t = wts.tile([P, n_dt, P], BF16, tag="uw")
                nc.sync.dma_start(uw_t[:], uw_d[:, ft])
                wt_tiles[ft] = (gw_t, uw_t)

            # Phase A: h[fp, ft, c] = silu(g) * u, F-major, bf16
            for ft in range(n_ft):
                if ft == 8:
                    # Down weights enqueued once phase A's own traffic has
                    # drained; ready well before phase B needs them.
                    nc.sync.dma_start(dw_t[:], dw_d[:])
                if ft not in wt_tiles:
                    load_ft(ft)
                gw_t, uw_t = wt_tiles.pop(ft)
                for (c0, cs) in achunks:
                    pg = ps.tile([P, 512], F32, tag="pg")
                    for dt_ in range(n_dt):
                        nc.tensor.matmul(
                            pg[:, :cs], gw_t[:, dt_], x_t[:, dt_, ds(c0, cs)],
                            start=(dt_ == 0), stop=(dt_ == n_dt - 1),
                        )
                    pu = ps.tile([P, 512], F32, tag="pu")
                    for dt_ in range(n_dt):
                        nc.tensor.matmul(
                            pu[:, :cs], uw_t[:, dt_], x_t[:, dt_, ds(c0, cs)],
                            start=(dt_ == 0), stop=(dt_ == n_dt - 1),
                        )
                    sg = stage.tile([P, 512], F32, tag="sg")
                    nc.scalar.activation(sg[:, :cs], pg[:, :cs],
                                         mybir.ActivationFunctionType.Silu)
                    nc.vector.tensor_tensor(
                        h_t[:, ft, ds(c0, cs)], sg[:, :cs], pu[:, :cs],
                        mybir.AluOpType.mult,
                    )

            # Phase B: out[d, c] = sum_f dwT[f, d] * h[f, c]
            # dw stationary (always 128 cols -> FWL), tokens moving, so the
            # trailing partial token tile costs only its actual width.
            for (c0, cs) in bchunks:
                for do in range(n_do):
                    po = ps.tile([P, 512], F32, tag="po")
                    for fo in range(n_ft):
                        nc.tensor.matmul(
                            po[:, :cs], dw_t[:, fo, ds(do * P, P)],
                            h_t[:, fo, ds(c0, cs)],
                            start=(fo == 0), stop=(fo == n_ft - 1),
                        )
                    osb = stage.tile([P, 512], BF16, tag="osb")
                    nc.scalar.activation(osb[:, :cs], po[:, :cs],
                                         mybir.ActivationFunctionType.Copy)
                    nc.sync.dma_start(out_d[do, :, ds(c0, cs)], osb[:, :cs])
    nc.finalize()
    return nc


def _route(gating_output):
    """Numpy softmax + top-2 + renormalize; returns (ids [T,K], w [T,K])."""
    g = gating_output.astype(np.float32)
    m = g.max(axis=-1, keepdims=True)
    e = np.exp(g - m)
    probs = e / e.sum(axis=-1, keepdims=True)
    ids = np.argsort(-probs, axis=-1, kind="stable")[:, :TOPK]
    w = np.take_along_axis(probs, ids, axis=-1)
    w = w / w.sum(axis=-1, keepdims=True)
    return ids, w


def kernel(x, gating_output, gate_w, up_w, down_w):
    x = np.asarray(x, dtype=np.float32)
    gating_output = np.asarray(gating_output, dtype=np.float32)
    gate_w = np.asarray(gate_w, dtype=np.float32)
    up_w = np.asarray(up_w, dtype=np.float32)
    down_w = np.asarray(down_w, dtype=np.float32)

    ids, w = _route(gating_output)

    idx_e = []
    w_e = []
    for e in range(E):
        sel = np.nonzero((ids == e).any(axis=-1))[0]
        kpos = (ids[sel] == e).argmax(axis=-1)
        idx_e.append(sel)
        w_e.append(w[sel, kpos])

    cap = max(len(i) for i in idx_e)
    cap += cap & 1

    nc = _build(cap)

    in_maps = []
    for e in range(E):
        idx = idx_e[e]
        cnt = len(idx)
        x_pad = np.zeros((cap, D), dtype=np.float32)
        x_pad[:cnt] = x[idx]

        # x: [cap, D] -> [128(dp), D/128(dt), cap]
        x_dev = np.ascontiguousarray(
            x_pad.T.reshape(D // P, P, cap).transpose(1, 0, 2).astype(NP_BF16))
        # gate/up: [F, D] -> T -> [D, F] -> [128(dp), 16(ft), 8(dt), 128(fi)]
        gwT = gate_w[e].T  # [D, F]
        gw_dev = np.ascontiguousarray(
            gwT.reshape(D // P, P, F // P, P).transpose(1, 2, 0, 3).astype(NP_BF16))
        uwT = up_w[e].T
        uw_dev = np.ascontiguousarray(
            uwT.reshape(D // P, P, F // P, P).transpose(1, 2, 0, 3).astype(NP_BF16))
        # down: [D, F] -> T -> [F, D] -> [128(fp), 16(fo), D]
        dwT = down_w[e].T  # [F, D]
        dw_dev = np.ascontiguousarray(
            dwT.reshape(F // P, P, D).transpose(1, 0, 2).astype(NP_BF16))
        gw0_dev = np.ascontiguousarray(gw_dev[:, 0])
        uw0_dev = np.ascontiguousarray(uw_dev[:, 0])

        in_maps.append({"x": x_dev, "gw": gw_dev, "uw": uw_dev, "dw": dw_dev,
                        "gw0": gw0_dev, "uw0": uw0_dev})

    try:
        res = run_bass_kernel_spmd(nc, in_maps, core_ids=list(range(N_CORES)))
    except Exception:
        # First execution of a fresh NEFF occasionally dies with
        # NRT_EXEC_UNIT_UNRECOVERABLE on this setup; the retry reuses the
        # cached executable and goes through.
        import time as _time

        _time.sleep(5)
        res = run_bass_kernel_spmd(nc, in_maps, core_ids=list(range(N_CORES)))

    out = np.zeros((T, D), dtype=np.float32)
    for e in range(E):
        cnt = len(idx_e[e])
        oe = res.results[e]["out"].reshape(D, cap).astype(np.float32)  # [d, token]
        out[idx_e[e]] += oe.T[:cnt] * w_e[e][:, None]
    return out


# revision 20
# speedup vs baseline: 1.0311x; 1.0084x over previous
"""Dense-MoE (top-2 of 8 experts) TRN2 kernel: expert-parallel over 8 NeuronCores.

Host side: softmax + top-2 routing, per-expert token gather (padded to the max
expert load), weight re-layout, fp32->bf16 cast. Device side (per core = one
expert), all matmul operands bf16 (PSUM fp32):
    h = silu(x_e @ gw.T) * (x_e @ uw.T)        [F-major in SBUF, bf16]
    out_e[d, t] = sum_f dwT[f, d] * h[f, t]     [dw stationary, tokens moving]
Host applies the routing weights and scatter-adds the 8 per-expert outputs
into the [T, D] result.

bf16 keeps absmax rel err ~4e-3 (vs the 2e-2 gate) while halving DMA traffic;
tokens-moving phase B avoids paying a full 512-col PE pass for the partial
last token tile.
"""
import sys

sys.path.insert(0, "/opt/trn_rl_repo")

import numpy as np
import ml_dtypes

import concourse.bass as bass
from concourse import bacc
import concourse.mybir as mybir
import concourse.tile as tile
from concourse.bass_utils import run_bass_kernel_spmd
from concourse.bass import ds

T, D, F, E, TOPK = 4096, 1024, 2048, 8, 2
P = 128
N_CORES = 8

F32 = mybir.dt.float32
BF16 = mybir.dt.bfloat16
NP_BF16 = ml_dtypes.bfloat16


def _chunks(cap):
    """Balanced even token chunks <=512."""
    n = -(-cap // 512)
    base = (cap // n) & ~1
    sizes = [base] * n
    rem = cap - base * n
    i = 0
    while rem > 0:
        sizes[i % n] += 2
        rem -= 2
        i += 1
    out = []
    c0 = 0
    for cs in sizes:
        out.append((c0, cs))
        c0 += cs
    return out


def _build(cap):
    """Per-core Bass program for capacity `cap` (even) tokens."""
    assert cap % 2 == 0
    achunks = _chunks(cap)            # phase A token chunks
    bchunks = _chunks(cap)            # phase B token chunks
    n_dt = D // P                     # 8 contraction slices
    n_ft = F // P                     # 16 f tiles
    n_do = D // P                     # 8 output d tiles

    nc = bacc.Bacc(None, target_bir_lowering=False)
    x_d = nc.declare_dram_parameter("x", [P, n_dt, cap], BF16, isOutput=False)
    gw_d = nc.declare_dram_parameter("gw", [P, n_ft, n_dt, P], BF16, isOutput=False)
    uw_d = nc.declare_dram_parameter("uw", [P, n_ft, n_dt, P], BF16, isOutput=False)
    # First f-tile of gate/up duplicated in tile layout (2KB/partition
    # contiguous) so the opening loads are single efficient DMAs.
    gw0_d = nc.declare_dram_parameter("gw0", [P, n_dt, P], BF16, isOutput=False)
    uw0_d = nc.declare_dram_parameter("uw0", [P, n_dt, P], BF16, isOutput=False)
    dw_d = nc.declare_dram_parameter("dw", [P, n_ft, D], BF16, isOutput=False)
    out_d = nc.declare_dram_parameter("out", [n_do, P, cap], BF16, isOutput=True)

    with tile.TileContext(nc) as tc:
        with (
            tc.tile_pool(name="deep", bufs=1) as deep,
            tc.tile_pool(name="wts", bufs=3) as wts,
            tc.tile_pool(name="stage", bufs=2) as stage,
            tc.tile_pool(name="ps", bufs=2, space="PSUM") as ps,
        ):
            wt_tiles = {}

            # Prologue: first-ft weights from the duplicated contiguous
            # copies, x as whole d-rows (2116B/partition bursts). The PE's
            # first matmul only waits on gw0 + the first x row.
            gw0 = wts.tile([P, n_dt, P], BF16, tag="gw")
            nc.sync.dma_start(gw0[:], gw0_d[:])
            x_t = deep.tile([P, n_dt, cap], BF16, tag="x")
            nc.sync.dma_start(x_t[:, 0], x_d[:, 0])
            nc.sync.dma_start(x_t[:, 1], x_d[:, 1])
            uw0 = wts.tile([P, n_dt, P], BF16, tag="uw")
            nc.sync.dma_start(uw0[:], uw0_d[:])
            for dt_ in range(2, n_dt):
                nc.sync.dma_start(x_t[:, dt_], x_d[:, dt_])
            wt_tiles[0] = (gw0, uw0)

            h_t = deep.tile([P, n_ft, cap], BF16, tag="h")
            dw_t = deep.tile([P, n_ft, D], BF16, tag="dw")

            def load_ft(ft):
                gw_t = wts.tile([P, n_dt, P], BF16, tag="gw")
                nc.sync.dma_start(gw_t[:], gw_d[:, ft])
                uw_t = wts.tile([P, n_dt, P], BF16, tag="uw")
                nc.sync.dma_start(uw_t[:], uw_d[:, ft])
                wt_tiles[ft] = (gw_t, uw_t)

            # Phase A: h[fp, ft, c] = silu(g) * u, F-major, bf16
            for ft in range(n_ft):
                if ft == 8:
                    # Down weights enqueued once phase A's own traffic has
                    # drained; ready well before phase B needs them.
                    nc.sync.dma_start(dw_t[:], dw_d[:])
                if ft not in wt_tiles:
                    load_ft(ft)
                gw_t, uw_t = wt_tiles.pop(ft)
                for (c0, cs) in achunks:
                    pg = ps.tile([P, 512], F32, tag="pg")
                    for dt_ in range(n_dt):
                        nc.tensor.matmul(
                            pg[:, :cs], gw_t[:, dt_], x_t[:, dt_, ds(c0, cs)],
                            start=(dt_ == 0), stop=(dt_ == n_dt - 1),
                        )
                    pu = ps.tile([P, 512], F32, tag="pu")
                    for dt_ in range(n_dt):
                        nc.tensor.matmul(
                            pu[:, :cs], uw_t[:, dt_], x_t[:, dt_, ds(c0, cs)],
                            start=(dt_ == 0), stop=(dt_ == n_dt - 1),
                        )
                    sg = stage.tile([P, 512], F32, tag="sg")
                    nc.scalar.activation(sg[:, :cs], pg[:, :cs],
                                         mybir.ActivationFunctionType.Silu)
                    nc.vector.tensor_tensor(
                        h_t[:, ft, ds(c0, cs)], sg[:, :cs], pu[:, :cs],
                        mybir.AluOpType.mult,
                    )

            # Phase B: out[d, c] = sum_f dwT[f, d] * h[f, c]
            # dw stationary (always 128 cols -> FWL), tokens moving, so the
            # trailing partial token tile costs only its actual width.
            for (c0, cs) in bchunks:
                for do in range(n_do):
                    po = ps.tile([P, 512], F32, tag="po")
                    for fo in range(n_ft):
                        nc.tensor.matmul(
                            po[:, :cs], dw_t[:, fo, ds(do * P, P)],
                            h_t[:, fo, ds(c0, cs)],
                            start=(fo == 0), stop=(fo == n_ft - 1),
                        )
                    osb = stage.tile([P, 512], BF16, tag="osb")
                    nc.scalar.activation(osb[:, :cs], po[:, :cs],
                                         mybir.ActivationFunctionType.Copy)
                    nc.sync.dma_start(out_d[do, :, ds(c0, cs)], osb[:, :cs])
    nc.finalize()
    return nc


def _route(gating_output):
    """Numpy softmax + top-2 + renormalize; returns (ids [T,K], w [T,K])."""
    g = gating_output.astype(np.float32)
    m = g.max(axis=-1, keepdims=True)
    e = np.exp(g - m)
    probs = e / e.sum(axis=-1, keepdims=True)
    ids = np.argsort(-probs, axis=-1, kind="stable")[:, :TOPK]
    w = np.take_along_axis(probs, ids, axis=-1)
    w = w / w.sum(axis=-1, keepdims=True)
    return ids, w


def kernel(x, gating_output, gate_w, up_w, down_w):
    x = np.asarray(x, dtype=np.float32)
    gating_output = np.asarray(gating_output, dtype=np.float32)
    gate_w = np.asarray(gate_w, dtype=np.float32)
    up_w = np.asarray(up_w, dtype=np.float32)
    down_w = np.asarray(down_w, dtype=np.float32)

    ids, w = _route(gating_output)

    idx_e = []
    w_e = []
    for e in range(E):
        sel = np.nonzero((ids == e).any(axis=-1))[0]
        kpos = (ids[sel] == e).argmax(axis=-1)
        idx_e.append(sel)
        w_e.append(w[sel, kpos])

    cap = max(len(i) for i in idx_e)
    cap += cap & 1

    nc = _build(cap)

    in_maps = []
    for e in range(E):
        idx = idx_e[e]
        cnt = len(idx)
        x_pad = np.zeros((cap, D), dtype=np.float32)
        x_pad[:cnt] = x[idx]

        # x: [cap, D] -> [128(dp), D/128(dt), cap]
        x_dev = np.ascontiguousarray(
            x_pad.T.reshape(D // P, P, cap).transpose(1, 0, 2).astype(NP_BF16))
        # gate/up: [F, D] -> T -> [D, F] -> [128(dp), 16(ft), 8(dt), 128(fi)]
        gwT = gate_w[e].T  # [D, F]
        gw_dev = np.ascontiguousarray(
            gwT.reshape(D // P, P, F // P, P).transpose(1, 2, 0, 3).astype(NP_BF16))
        uwT = up_w[e].T
        uw_dev = np.ascontiguousarray(
            uwT.reshape(D // P, P, F // P, P).transpose(1, 2, 0, 3).astype(NP_BF16))
        # down: [D, F] -> T -> [F, D] -> [128(fp), 16(fo), D]
        dwT = down_w[e].T  # [F, D]
        dw_dev = np.ascontiguousarray(
            dwT.reshape(F // P, P, D).transpose(1, 0, 2).astype(NP_BF16))
        gw0_dev = np.ascontiguousarray(gw_dev[:, 0])
        uw0_dev = np.ascontiguousarray(uw_dev[:, 0])

        in_maps.append({"x": x_dev, "gw": gw_dev, "uw": uw_dev, "dw": dw_dev,
                        "gw0": gw0_dev, "uw0": uw0_dev})

    try:
        res = run_bass_kernel_spmd(nc, in_maps, core_ids=list(range(N_CORES)))
    except Exception:
        # First execution of a fresh NEFF occasionally dies with
        # NRT_EXEC_UNIT_UNRECOVERABLE on this setup; the retry reuses the
        # cached executable and goes through.
        import time as _time

        _time.sleep(5)
        res = run_bass_kernel_spmd(nc, in_maps, core_ids=list(range(N_CORES)))

    out = np.zeros((T, D), dtype=np.float32)
    for e in range(E):
        cnt = len(idx_e[e])
        oe = res.results[e]["out"].reshape(D, cap).astype(np.float32)  # [d, token]
        out[idx_e[e]] += oe.T[:cnt] * w_e[e][:, None]
    return out
